# revision 36
# baseline (speedup 1.0000x reference)
"""Multi-head self-attention (B=2, S=2048, D=1024, H=16) on 8 TRN2 NeuronCores.

Sharding: batch*heads tensor-parallel. Each core owns 2 heads (both batches):
QKV projection for its heads only (W_qkv output-dim sharded), full attention
for its 2x2 (batch, head) pairs, partial output projection (W_out input-dim
sharded). Partials are summed on the host (the "all-reduce") + output bias.

The kernel is ACT-bound: exp over the full score matrix is 16.8M elems/core
= 128 activations of [128, 1024] ~ 1.1us each ~ 140us. Everything else is
structured to hide under that stream:

  - P2 runs in quarter-passes (512 q-tokens, both heads packed side by side
    in ONE [128, 1024] PSUM scores tile, double-buffered) so each ki step is
    a single exp and the PE can run 1-2 ki ahead of the ACT.
    PSUM: scores 2x2 banks + 2 AV accumulator banks + 2 work banks = 8.
  - P1(b1) (QKV matmuls) is emitted in small units between the ki steps of
    P2(b0); P3 for each quarter is emitted between the ki steps of the NEXT
    quarter-pass. The PE never idles >3.4us, so the HAM clock gate stays at
    2.4 GHz; only P1(b0) at the start and the last quarter's P3 are serial.
  - v is transposed into the AV lhsT layout by the DMA xbar
    (dma_start_transpose), not the PE/DVE.
  - AV accumulators [65, 512] carry the softmax denominator in row 64 (vaug
    ones-column trick). DVE reciprocal cost is ~6.4ns per FREE-dim element
    regardless of partition count, so denominators are PE-transposed into a
    [128, 8] column block, reciprocal'd there (~0.2us), and PE-transposed
    back into rows for the [64, 512] broadcast matmul that feeds the
    normalize multiply (which reads the broadcast straight from PSUM).
  - Output is written bf16 (halves the store traffic; host accumulates).
"""

import sys

for _p in ("/opt/trn_rl_repo", "/root/.axon_site/_ro/trn_rl_repo"):
    if _p not in sys.path:
        sys.path.insert(0, _p)

from contextlib import ExitStack

import numpy as np

import concourse.bacc as bacc
import concourse.bass as bass
import concourse.mybir as mybir
import concourse.tile as tile
from concourse.bass_utils import run_bass_kernel_spmd
from concourse.masks import make_identity

F32 = mybir.dt.float32
F32R = mybir.dt.float32r
BF16 = mybir.dt.bfloat16

B, S, D, H = 2, 2048, 1024, 16
HD = D // H  # 64
T = B * S  # 4096 tokens
SCALE = HD**-0.5
N_CORES = 8
HEADS_PER_CORE = H // N_CORES  # 2

EXP = mybir.ActivationFunctionType.Exp
COPY_FN = mybir.ActivationFunctionType.Identity
USE_DMA_TRANSPOSE = True
# vaug column layout: h0 v @ 0:64, ones @ 64; h1 v @ 80:144, ones @ 144.
# (h1's v block starts at byte offset 160, 32B-aligned for the DMA xbar.)
VA_W = 146
VA_OFF = (0, 80)


def build_kernel() -> bacc.Bacc:
    nc = bacc.Bacc(target_bir_lowering=False)
    xp = nc.dram_tensor("xp", [8, 128, 8 * 512], BF16, kind="ExternalInput")
    wqkvp = nc.dram_tensor("wqkvp", [128, 8 * 6 * HD], BF16, kind="ExternalInput")
    woutT = nc.dram_tensor("woutT", [2 * HD, D], BF16, kind="ExternalInput")
    out = nc.dram_tensor("out", [T, D], BF16, kind="ExternalOutput")

    with tile.TileContext(nc) as tc, ExitStack() as ctx:
        const = ctx.enter_context(tc.tile_pool(name="const", bufs=1))
        sb = ctx.enter_context(tc.tile_pool(name="sb", bufs=1))
        ps = ctx.enter_context(tc.tile_pool(name="ps", bufs=1, space="PSUM"))

        # x chunk 0 DMA first: P1(b0) compute depends on it.
        x_tiles = {}

        def load_x(b, ch):
            def run():
                x_sb = sb.tile([128, 8, 512], BF16, tag="x", bufs=6, name=f"x{b}{ch}")
                # b1's prefetch goes on the scalar-issued DMA queue so the big
                # transfers don't sit ahead of the vaug xbar transposes on the
                # sync queue.
                eng = nc.scalar if b == 1 else nc.sync
                eng.dma_start(
                    out=x_sb,
                    in_=xp[4 * b + ch].rearrange("p (t n) -> p t n", t=8),
                )
                x_tiles[b, ch] = x_sb

            return run

        load_x(0, 0)()

        ident = const.tile([128, 128], BF16)
        make_identity(nc, ident)
        identf = const.tile([128, 128], F32)
        make_identity(nc, identf)
        ones64_f32 = const.tile([1, 64], F32)
        nc.vector.memset(ones64_f32, 1.0)
        ones64 = const.tile([1, 64], F32R)
        nc.vector.tensor_copy(ones64[:], ones64_f32[:])
        ones_col = const.tile([128, 1], BF16)
        nc.vector.memset(ones_col, 1.0)

        w_sb = const.tile([128, 8, 6 * HD], BF16)
        nc.sync.dma_start(out=w_sb, in_=wqkvp.rearrange("p (t c) -> p t c", t=8))
        wo = const.tile([2 * HD, D], BF16)
        wo_h1 = const.tile([HD, D], BF16)

        def load_wo():
            nc.sync.dma_start(out=wo, in_=woutT[:, :])
            nc.sync.dma_start(out=wo_h1, in_=woutT[HD:, :])

        # persistent SBUF state
        qT = {b: sb.tile([128, S], BF16, tag="qk", bufs=4, name=f"qT{b}") for b in range(B)}
        kT = {b: sb.tile([128, S], BF16, tag="qk", bufs=4, name=f"kT{b}") for b in range(B)}
        vaug = {
            b: [sb.tile([128, VA_W], BF16, tag="vaug", bufs=32, name=f"va{b}_{t}") for t in range(16)]
            for b in range(B)
        }
        acc_all = {
            b: sb.tile([65, 4096], BF16, tag="acc", bufs=2, name=f"acc{b}") for b in range(B)
        }
        rec = {b: sb.tile([1, 4096], F32R, tag="rec", bufs=2, name=f"rec{b}") for b in range(B)}
        oTn = {b: sb.tile([128, S], BF16, tag="ot", bufs=2, name=f"oTn{b}") for b in range(B)}

        # ------------------------------------------------------------------
        # P1 units: QKV projection (2 matmuls per unit) + v DMA-transposes.
        # ------------------------------------------------------------------
        def p1_units(b, with_wo_dma=False):
            units = []
            vT = sb.tile([128, S], BF16, tag="vt", bufs=2, name=f"vT{b}")
            state = {}

            def mm_part(ch, g, part, dst):
                def run():
                    x_sb = x_tiles[b, ch]
                    if part == 0:
                        state["acc"] = ps.tile(
                            [128, 512], F32, tag="work", bufs=2, name="qkv"
                        )
                    acc = state["acc"]
                    for t in range(2 * part, 2 * part + 2):
                        nc.tensor.matmul(
                            acc[:],
                            w_sb[:, t, g * 128 : (g + 1) * 128],
                            x_sb[:, t, :],
                            start=(t == 0),
                            stop=(t == 7),
                        )
                    if part == 3:
                        csl = slice(ch * 512, (ch + 1) * 512)
                        nc.vector.tensor_copy(dst[:, csl], acc[:])

                return run

            def vtrans(ti):
                def run():
                    va = vaug[b][ti]
                    tsl = slice(ti * 128, (ti + 1) * 128)
                    o0, o1 = VA_OFF
                    if USE_DMA_TRANSPOSE:
                        nc.sync.dma_start_transpose(va[:, o0 : o0 + 64], vT[0:64, tsl])
                        nc.sync.dma_start_transpose(va[:, o1 : o1 + 64], vT[64:128, tsl])
                    else:
                        tp = ps.tile([128, 128], BF16, tag="work", bufs=2, name="trps")
                        nc.tensor.transpose(tp[:], vT[:, tsl], ident[:])
                        nc.vector.tensor_copy(va[:, o0 : o0 + 64], tp[:, 0:64])
                        nc.vector.tensor_copy(va[:, o1 : o1 + 64], tp[:, 64:128])
                    nc.gpsimd.memset(va[:, o0 + 64 : o0 + 65], 1.0)
                    nc.gpsimd.memset(va[:, o1 + 64 : o1 + 65], 1.0)

                return run

            if b > 0:
                for ch in range(4):
                    units.append(load_x(b, ch))
            else:
                for ch in range(1, 4):
                    units.append(load_x(b, ch))
            for ch in range(4):
                for g, dst in ((0, qT[b]), (1, kT[b]), (2, vT)):
                    for part in range(4):
                        units.append(mm_part(ch, g, part, dst))
                if ch == 0 and with_wo_dma:
                    units.append(load_wo)
                for ti in range(4 * ch, 4 * ch + 4):
                    units.append(vtrans(ti))
            return units

        # ------------------------------------------------------------------
        # P3 units for (batch, quarter): transpose-dance reciprocal, PE
        # broadcast + normalize, output projection + staging + store.
        # ------------------------------------------------------------------
        def p3_units(b, q):
            units = []
            state = {}

            def dance_fwd():
                dn_t = ps.tile([128, 8], F32, tag="work", bufs=2, name="dnps")
                for j in range(8):
                    c0 = q * 1024 + j * 128
                    # [1,128] row -> [128,1] column is a K=1 matmul vs ones
                    nc.tensor.matmul(
                        dn_t[:, j : j + 1],
                        acc_all[b][64:65, c0 : c0 + 128],
                        ones_col[64:65, :],
                        start=True,
                        stop=True,
                    )
                state["dn_t"] = dn_t

            def dance_recip():
                dn_s = sb.tile([128, 8], F32, tag="dns", bufs=2, name="dns")
                nc.vector.tensor_copy(dn_s[:], state["dn_t"][:])
                rec_t = sb.tile([128, 8], F32, tag="rect", bufs=2, name="rect")
                with nc.allow_low_precision(reason="softmax denom recip"):
                    nc.vector.reciprocal(rec_t[:], dn_s[:])
                state["rec_t"] = rec_t

            def dance_back(h):
                def run():
                    rp = ps.tile([1, 512], F32, tag="work", bufs=2, name="rpps")
                    for j in range(4):
                        c = 4 * h + j
                        nc.tensor.transpose(
                            rp[:, j * 128 : (j + 1) * 128],
                            state["rec_t"][:, c : c + 1],
                            identf[:],
                        )
                    c0 = (2 * q + h) * 512
                    nc.vector.tensor_copy(rec[b][:, c0 : c0 + 512], rp[:])

                return run

            units += [dance_fwd, dance_recip, dance_back(0), dance_back(1)]

            def norm_u(h):
                def run():
                    c0 = (2 * q + h) * 512
                    bc = ps.tile([64, 512], F32, tag="work", bufs=2, name="bcps")
                    nc.tensor.matmul(
                        bc[:], ones64[:], rec[b][:, c0 : c0 + 512], start=True, stop=True
                    )
                    nc.vector.tensor_mul(
                        oTn[b][64 * h : 64 * h + 64, q * 512 : (q + 1) * 512],
                        acc_all[b][0:64, c0 : c0 + 512],
                        bc[:],
                    )

                return run

            units.append(norm_u(0))
            units.append(norm_u(1))

            def proj_u(tc_i, nk):
                def run():
                    tsl = slice(tc_i * 128, (tc_i + 1) * 128)
                    nsl = slice(nk * 512, (nk + 1) * 512)
                    op = ps.tile([128, 512], F32, tag="work", bufs=2, name="outps")
                    nc.tensor.matmul(op[:], oTn[b][:, tsl], wo[:, nsl], start=True, stop=True)
                    if nk == 0:
                        state["ob"] = sb.tile([128, D], BF16, tag="outsb", bufs=3, name="ob")
                    ob = state["ob"]
                    nc.vector.tensor_copy(ob[:, nsl], op[:])
                    if nk == 1:
                        r0 = b * S + tc_i * 128
                        nc.sync.dma_start(out=out[r0 : r0 + 128, :], in_=ob[:])

                return run

            for tc_i in range(4 * q, 4 * q + 4):
                units.append(proj_u(tc_i, 0))
                units.append(proj_u(tc_i, 1))
            return units

        # ------------------------------------------------------------------
        # P2 quarter-pass (b, q): 16 ki steps, each = 2 scores MMs (one per
        # head, concurrent PE quadrants) + 1 exp [128,1024] + 2 AV MMs (for
        # the previous ki). `slots[ki]` = interleave thunks after step ki.
        # ------------------------------------------------------------------
        def p2_pass(b, q, slots):
            qsl = slice(q * 512, (q + 1) * 512)
            accs = {
                h: ps.tile([65, 512], F32, tag="av", bufs=2, name=f"av{b}{q}{h}")
                for h in range(2)
            }
            prev = None
            for ki in range(16):
                ksl = slice(ki * 128, (ki + 1) * 128)
                scs = ps.tile([128, 1024], F32, tag="sc", bufs=2, name="scps")
                prs = sb.tile([128, 1024], BF16, tag="pr", bufs=3, name="pr")
                for h in range(2):
                    p0 = h * 64
                    nc.tensor.matmul(
                        scs[:, h * 512 : (h + 1) * 512],
                        kT[b][p0 : p0 + 64, ksl],
                        qT[b][p0 : p0 + 64, qsl],
                        start=True,
                        stop=True,
                    )
                nc.scalar.activation(prs[:], scs[:], EXP, scale=SCALE)
                if prev is not None:
                    _av(nc, accs, vaug[b], prev[0], prev[1])
                prev = (prs, ki)
                for thunk in slots[ki]:
                    thunk()
            _av(nc, accs, vaug[b], prev[0], prev[1])
            for h in range(2):
                c0 = (2 * q + h) * 512
                nc.vector.tensor_copy(acc_all[b][:, c0 : c0 + 512], accs[h][:])

        def spread(units, n=16, first=1):
            per = [[] for _ in range(n)]
            m = n - first
            for i, u in enumerate(units):
                per[first + min(i * m // max(len(units), 1), m - 1)].append(u)
            return per

        # ---------------- emission ----------------
        for u in p1_units(0):
            u()

        # pass schedule: window B (passes 0-3) carries P1(b1); window C
        # (passes 4-7) carries the P3 quarters; b1's last quarter is the tail.
        p1b1 = p1_units(1, with_wo_dma=True)
        n_b = len(p1b1)
        sched = {4: [(0, 0), (0, 1)], 5: [(0, 2), (0, 3)], 6: [(1, 0), (1, 1)], 7: [(1, 2)]}
        for p in range(8):
            b, q = divmod(p, 4)
            extras = []
            for pb, pq in sched.get(p, []):
                extras += p3_units(pb, pq)
            if p < 4:
                lo = p * n_b // 4
                hi = (p + 1) * n_b // 4
                extras = p1b1[lo:hi] + extras
            p2_pass(b, q, spread(extras))

        # tail: P3(b1, q=3) — ACT is idle here, so normalize with
        # per-partition activation scale on token-major projection output
        # instead of the dance-back + broadcast-matmul path.
        b, q = 1, 3
        dn_t = ps.tile([128, 8], F32, tag="work", bufs=2, name="dnps")
        for j in range(8):
            c0 = q * 1024 + j * 128
            nc.tensor.matmul(
                dn_t[:, j : j + 1],
                acc_all[b][64:65, c0 : c0 + 128],
                ones_col[64:65, :],
                start=True,
                stop=True,
            )
        dn_s = sb.tile([128, 8], F32, tag="dns", bufs=2, name="dns")
        nc.vector.tensor_copy(dn_s[:], dn_t[:])
        rec_t = sb.tile([128, 8], F32, tag="rect", bufs=2, name="rect")
        with nc.allow_low_precision(reason="softmax denom recip"):
            nc.vector.reciprocal(rec_t[:], dn_s[:])
        for tc_j in range(4):
            ob = sb.tile([128, D], BF16, tag="outsb", bufs=3, name="obt")
            ts = []
            for h in range(2):
                c0 = (2 * q + h) * 512 + tc_j * 128
                # P2's PSUM is idle at the tail: use the 2-bank sc slots for
                # [128,1024] ops so each head is ONE wide ACT-scaled copy.
                op = ps.tile([128, D], F32, tag="sc", bufs=2, name="outps")
                for nk in range(2):
                    nsl = slice(nk * 512, (nk + 1) * 512)
                    nc.tensor.matmul(
                        op[:, nsl],
                        acc_all[b][0:64, c0 : c0 + 128],
                        (wo[0:64, nsl] if h == 0 else wo_h1[:, nsl]),
                        start=True,
                        stop=True,
                    )
                t = sb.tile([128, D], F32, tag="tt", bufs=4, name="tt")
                nc.scalar.activation(
                    t[:], op[:], COPY_FN, scale=rec_t[:, 4 * h + tc_j : 4 * h + tc_j + 1]
                )
                ts.append(t)
            nc.vector.tensor_add(ob[:], ts[0][:], ts[1][:])
            r0 = b * S + (4 * q + tc_j) * 128
            nc.sync.dma_start(out=out[r0 : r0 + 128, :], in_=ob[:])

    nc.finalize()
    return nc


def _av(nc, accs, vaug_b, prs, ki):
    for h in range(2):
        o = VA_OFF[h]
        nc.tensor.matmul(
            accs[h][:],
            vaug_b[ki][:, o : o + 65],
            prs[:, h * 512 : (h + 1) * 512],
            start=(ki == 0),
            stop=(ki == 15),
        )


_NC_CACHE = None
TRACE = False
LAST_RESULT = None


def _get_nc():
    global _NC_CACHE
    if _NC_CACHE is None:
        _NC_CACHE = build_kernel()
    return _NC_CACHE


def kernel(x, W_qkv, W_out, b_out):
    import ml_dtypes

    x = np.asarray(x, dtype=np.float32)
    W_qkv = np.asarray(W_qkv, dtype=np.float32)
    W_out = np.asarray(W_out, dtype=np.float32)
    b_out = np.asarray(b_out, dtype=np.float32)

    xTf = x.reshape(T, D).T  # [D, T]
    # per-(batch, chunk) contiguous blocks: [8, 128, 8*512], block = chunk's
    # [128 partitions, 8 t-subtiles, 512 tokens]
    xp = np.empty((8, 128, 8 * 512), dtype=ml_dtypes.bfloat16)
    for b in range(B):
        for ch in range(4):
            tok0 = b * S + ch * 512
            blk = xTf[:, tok0 : tok0 + 512].reshape(8, 128, 512).transpose(1, 0, 2)
            xp[4 * b + ch] = blk.reshape(128, 8 * 512).astype(ml_dtypes.bfloat16)
    in_maps = []
    for c in range(N_CORES):
        h0 = c * HEADS_PER_CORE
        rows = slice(h0 * HD, (h0 + 2) * HD)
        wq = W_qkv[0 * D :][rows]
        wk = W_qkv[1 * D :][rows]
        wv = W_qkv[2 * D :][rows]
        wqkvT = np.concatenate([wq, wk, wv], axis=0).T  # [D, 384]
        wqkvp = np.ascontiguousarray(
            wqkvT.reshape(8, 128, 6 * HD).transpose(1, 0, 2).reshape(128, 8 * 6 * HD)
        ).astype(ml_dtypes.bfloat16)
        woutT = np.ascontiguousarray(W_out[:, h0 * HD : (h0 + 2) * HD].T).astype(
            ml_dtypes.bfloat16
        )
        in_maps.append({"xp": xp, "wqkvp": wqkvp, "woutT": woutT})

    nc = _get_nc()
    global LAST_RESULT
    res = run_bass_kernel_spmd(nc, in_maps, core_ids=list(range(N_CORES)), trace=TRACE)
    LAST_RESULT = res
    partial = np.zeros((T, D), dtype=np.float64)
    for c in range(N_CORES):
        partial += res.results[c]["out"].astype(np.float64)
    full = (partial + b_out.astype(np.float64)).astype(np.float32)
    return full.reshape(B, S, D)
